# revision 9
# baseline (speedup 1.0000x reference)
"""Cross-batch attention (CAEncoder training path) on 8 trn2 NeuronCores.

Full inputs in, full outputs out.  Sharding: video batch axis (B=32) split
4-per-core across 8 cores; text_embd + mask are replicated to every core.
Each core computes its [4, 32, ...] slab of all four outputs.

Per-core bass/Tile kernel, per (v, text-pair i) iteration (t0=2i, t1=2i+1):
  ST  [128,(m=512)]   = (tT2_i).T @ (vT_scaled_v)     4 accumulating matmuls
                        partitions = (two,n) stacked pair of texts, f32r inputs
  ET  = exp(ST)  (ScalarE, fused accum_out -> text softmax denominators)
  A_text = ET * 1/sum  -> text_att_scores out  (full fp32)
  EvT = ET * maskcol   (per-partition {0,1} mask, f32r copy -> vis matmul lhsT)
  E2  = transpose(ET)  (4 TensorE transposes -> [m, (two,n)] layout, PSUM)
  Evmn = E2 * mask_rep (fp32, -> vis_att_scores out + row sums on DVE)
  text_embd = (E2_f32r).T @ video_nat  (4 accumulating f32r matmuls) * 1/sum
  video_embd = (EvT slice).T @ text_nat (8 f32r matmuls, K=64 row-packed) * 1/sum
"""

import math

import numpy as np

import concourse.bass as bass
import concourse.mybir as mybir
import concourse.tile as tile
from concourse import bacc
from concourse.bass_utils import run_bass_kernel_spmd
from concourse.masks import make_identity

B, NV, NT, D = 32, 512, 64, 512
NCORES = 8
VPC = B // NCORES          # videos per core = 4
NI = B // 2                # paired-text iterations = 16
P = 128
MC = NV // P               # m chunks = 4
DC = D // P                # d chunks = 4
F32 = mybir.dt.float32
F32R = mybir.dt.float32r

_PROGRAM = None


def _build_program():
    nc = bacc.Bacc(None)

    vns = nc.dram_tensor("vns", [VPC, NV, D], F32, kind="ExternalInput")
    vts = nc.dram_tensor("vts", [VPC, D, NV], F32, kind="ExternalInput")
    tn2 = nc.dram_tensor("tn2", [NI, P, D], F32, kind="ExternalInput")
    tt2 = nc.dram_tensor("tt2", [NI, D, 2 * NT], F32, kind="ExternalInput")
    mkcol = nc.dram_tensor("mkcol", [P, NI], F32, kind="ExternalInput")
    mkrep = nc.dram_tensor("mkrep", [NI, P, 2 * NT], F32, kind="ExternalInput")

    vout = nc.dram_tensor("vout", [VPC, B, NV, D], F32, kind="ExternalOutput")
    tout = nc.dram_tensor("tout", [VPC, B, NT, D], F32, kind="ExternalOutput")
    aout = nc.dram_tensor("aout", [VPC, B, NV, NT], F32, kind="ExternalOutput")
    bout = nc.dram_tensor("bout", [VPC, B, NT, NV], F32, kind="ExternalOutput")

    with tile.TileContext(nc) as tc:
        with (
            tc.tile_pool(name="consts", bufs=1) as consts,
            tc.tile_pool(name="work", bufs=2) as work,
            tc.tile_pool(name="outp", bufs=2) as outp,
            tc.tile_pool(name="scal", bufs=4) as scal,
            tc.tile_pool(name="ps_st", bufs=2, space="PSUM") as ps_st,
            tc.tile_pool(name="ps_e", bufs=2, space="PSUM") as ps_e,
            tc.tile_pool(name="ps_te", bufs=2, space="PSUM") as ps_te,
            tc.tile_pool(name="ps_ve", bufs=2, space="PSUM") as ps_ve,
        ):
            # ---- resident inputs (matmul operands hardware-rounded to f32r) ----
            # Split into per-v / per-i chunk DMAs, first-iteration deps first,
            # so compute starts ~3 small DMAs in instead of after a ~50us
            # serial prologue.
            vns_sb = consts.tile([P, VPC, MC, D], F32R)
            vts_sb = consts.tile([P, VPC, DC, NV], F32R)
            tn2_sb = consts.tile([P, NI, D], F32R)
            tt2_sb = consts.tile([P, NI, DC, 2 * NT], F32R)
            mkcol_sb = consts.tile([P, NI], F32)
            mkrep_sb = consts.tile([P, NI, 2 * NT], F32)
            ident = consts.tile([P, P], F32)

            def load_vts(v):
                nc.gpsimd.dma_start(
                    out=vts_sb[:, v], in_=vts[v].rearrange("(dc p) m -> p dc m", p=P)
                )

            def load_vns(v):
                nc.gpsimd.dma_start(
                    out=vns_sb[:, v], in_=vns[v].rearrange("(mc p) d -> p mc d", p=P)
                )

            def load_tt2(i):
                nc.gpsimd.dma_start(
                    out=tt2_sb[:, i], in_=tt2[i].rearrange("(dc p) j -> p dc j", p=P)
                )

            def load_tn2(i):
                nc.gpsimd.dma_start(out=tn2_sb[:, i], in_=tn2[i])

            load_vts(0)
            load_tt2(0)
            nc.sync.dma_start(out=mkcol_sb, in_=mkcol[:, :])
            make_identity(nc, ident)
            load_vns(0)
            load_tn2(0)
            nc.sync.dma_start(out=mkrep_sb, in_=mkrep.rearrange("i p j -> p i j"))
            for i in range(1, NI):
                load_tt2(i)
                load_tn2(i)
            for v in range(1, VPC):
                load_vts(v)
                load_vns(v)

            for v in range(VPC):
                for i in range(NI):
                    # ---- scores (transposed layout): ST[(two,n), m] ----
                    ST = ps_st.tile([P, NV], F32, tag="st")
                    for dc in range(DC):
                        nc.tensor.matmul(
                            ST,
                            lhsT=tt2_sb[:, i, dc, :],
                            rhs=vts_sb[:, v, dc, :],
                            start=(dc == 0),
                            stop=(dc == DC - 1),
                        )

                    # ---- exp + fused text-softmax denominators ----
                    ET = work.tile([P, NV], F32, tag="et")
                    stx = scal.tile([P, 1], F32, tag="stx")
                    nc.scalar.activation(
                        ET, ST, mybir.ActivationFunctionType.Exp, accum_out=stx
                    )
                    rtx = scal.tile([P, 1], F32, tag="rtx")
                    nc.vector.reciprocal(rtx, stx)

                    # ---- text_att_scores (full fp32) ----
                    At2 = outp.tile([P, NV], F32, tag="at2")
                    nc.vector.tensor_scalar_mul(At2, ET, rtx)
                    nc.sync.dma_start(
                        out=bout[v, 2 * i : 2 * i + 2].rearrange("t n m -> (t n) m"),
                        in_=At2,
                    )

                    # ---- masked exp (vis path), transposed layout, f32r ----
                    EvT = work.tile([P, NV], F32R, tag="evt")
                    nc.vector.tensor_scalar_mul(EvT, ET, mkcol_sb[:, i : i + 1])

                    # ---- transpose ET -> E2[m_local, mc, (two,n)] in PSUM ----
                    E2ps = ps_e.tile([P, MC, P], F32, tag="e2ps")
                    for mc in range(MC):
                        nc.tensor.transpose(
                            E2ps[:, mc, :], ET[:, mc * P : (mc + 1) * P], ident
                        )
                    E2r = work.tile([P, MC, P], F32R, tag="e2r")
                    nc.vector.tensor_copy(E2r, E2ps)

                    # ---- vis_att_scores numerators (fp32) + row sums ----
                    Evmn = work.tile([P, MC, 2, NT], F32, tag="evmn")
                    nc.vector.tensor_tensor(
                        Evmn,
                        E2ps[:].rearrange("p a (b c) -> p a b c", c=NT),
                        mkrep_sb[:, i, None, :]
                        .rearrange("p a (b c) -> p a b c", c=NT)
                        .to_broadcast([P, MC, 2, NT]),
                        op=mybir.AluOpType.mult,
                    )
                    sv = scal.tile([P, MC, 2], F32, tag="sv")
                    nc.vector.tensor_reduce(
                        sv, Evmn, axis=mybir.AxisListType.X, op=mybir.AluOpType.add
                    )
                    rv = scal.tile([P, MC, 2], F32, tag="rv")
                    nc.vector.reciprocal(rv, sv)

                    Avis = outp.tile([P, MC, 2, NT], F32, tag="avis")
                    nc.vector.tensor_tensor(
                        Avis,
                        Evmn,
                        rv[:, :, :, None].to_broadcast([P, MC, 2, NT]),
                        op=mybir.AluOpType.mult,
                    )
                    for two in range(2):
                        nc.sync.dma_start(
                            out=aout[v, 2 * i + two].rearrange(
                                "(mc p) n -> p mc n", p=P
                            ),
                            in_=Avis[:, :, two, :],
                        )

                    # ---- text_embd_att ----
                    TE = ps_te.tile([P, D], F32, tag="te")
                    for mc in range(MC):
                        nc.tensor.matmul(
                            TE,
                            lhsT=E2r[:, mc, :],
                            rhs=vns_sb[:, v, mc, :],
                            start=(mc == 0),
                            stop=(mc == MC - 1),
                        )
                    te2 = outp.tile([P, D], F32, tag="te2")
                    nc.scalar.activation(
                        te2, TE, mybir.ActivationFunctionType.Copy, scale=rtx
                    )
                    nc.sync.dma_start(
                        out=tout[v, 2 * i : 2 * i + 2].rearrange("t n d -> (t n) d"),
                        in_=te2,
                    )

                    # ---- video_embd_att: 8 K=64 f32r matmuls (row-packed) ----
                    for two in range(2):
                        veb = outp.tile([P, MC, D], F32, tag="vebig")
                        pb = two * NT
                        for mc in range(MC):
                            VE = ps_ve.tile([P, D], F32, tag="ve")
                            nc.tensor.matmul(
                                VE,
                                lhsT=EvT[pb : pb + NT, mc * P : (mc + 1) * P],
                                rhs=tn2_sb[pb : pb + NT, i, :],
                                start=True,
                                stop=True,
                                tile_position=(pb, 0),
                            )
                            rv_s = rv[:, mc, two : two + 1]
                            if mc % 2 == 0:
                                nc.scalar.activation(
                                    veb[:, mc, :],
                                    VE,
                                    mybir.ActivationFunctionType.Copy,
                                    scale=rv_s,
                                )
                            else:
                                nc.vector.tensor_scalar_mul(veb[:, mc, :], VE, rv_s)
                        nc.sync.dma_start(
                            out=vout[v, 2 * i + two].rearrange(
                                "(mc p) d -> p mc d", p=P
                            ),
                            in_=veb,
                        )
    nc.finalize()
    return nc


def _get_program():
    global _PROGRAM
    if _PROGRAM is None:
        _PROGRAM = _build_program()
    return _PROGRAM


def _prep_in_maps(video_embd, text_embd, mask):
    video = np.ascontiguousarray(np.asarray(video_embd, dtype=np.float32))
    text = np.ascontiguousarray(np.asarray(text_embd, dtype=np.float32))
    maskf = np.asarray(mask).astype(np.float32)

    scale = np.float32(1.0) / np.sqrt(np.float32(D))

    tn2 = np.ascontiguousarray(text.reshape(NI, P, D))
    tt2 = np.ascontiguousarray(
        text.reshape(NI, 2, NT, D).transpose(0, 3, 1, 2).reshape(NI, D, 2 * NT)
    )
    mkcol = np.ascontiguousarray(maskf.reshape(NI, 2 * NT).T)
    mkrep = np.ascontiguousarray(
        np.broadcast_to(maskf.reshape(NI, 1, 2 * NT), (NI, P, 2 * NT))
    )

    in_maps = []
    for c in range(NCORES):
        sl = video[c * VPC : (c + 1) * VPC]
        in_maps.append(
            {
                "vns": sl,
                "vts": np.ascontiguousarray(sl.transpose(0, 2, 1)) * scale,
                "tn2": tn2,
                "tt2": tt2,
                "mkcol": mkcol,
                "mkrep": mkrep,
            }
        )
    return in_maps


def _run(video_embd, text_embd, mask, trace=False):
    nc = _get_program()
    in_maps = _prep_in_maps(video_embd, text_embd, mask)
    out = run_bass_kernel_spmd(nc, in_maps, list(range(NCORES)), trace=trace)
    res = out.results
    video_embd_att = np.concatenate([res[c]["vout"] for c in range(NCORES)], axis=0)
    text_embd_att = np.concatenate([res[c]["tout"] for c in range(NCORES)], axis=0)
    vis_att_scores = np.concatenate([res[c]["aout"] for c in range(NCORES)], axis=0)
    text_att_scores = np.concatenate([res[c]["bout"] for c in range(NCORES)], axis=0)
    return (
        (video_embd_att, text_embd_att, vis_att_scores, text_att_scores),
        out.exec_time_ns,
    )


def kernel(video_embd, text_embd, mask):
    outs, _ = _run(video_embd, text_embd, mask, trace=False)
    return outs


# revision 10
# speedup vs baseline: 8.4481x; 8.4481x over previous
"""Cross-batch attention (CAEncoder training path) on 8 trn2 NeuronCores.

Full inputs in, full outputs out.  Sharding: video batch axis (B=32) split
4-per-core across 8 cores; text_embd + mask are replicated to every core.
Each core computes its [4, 32, ...] slab of all four outputs.

Per-core bass/Tile kernel, per (v, text-pair i) iteration (t0=2i, t1=2i+1):
  ST  [128,(m=512)]   = (tT2_i).T @ (vT_scaled_v)     4 accumulating matmuls
                        partitions = (two,n) stacked pair of texts, f32r inputs
  ET  = exp(ST)  (ScalarE, fused accum_out -> text softmax denominators)
  A_text = ET * 1/sum  -> text_att_scores out  (full fp32)
  EvT = ET * maskcol   (per-partition {0,1} mask, f32r copy -> vis matmul lhsT)
  E2  = transpose(ET)  (4 TensorE transposes -> [m, (two,n)] layout, PSUM)
  Evmn = E2 * mask_rep (fp32, -> vis_att_scores out + row sums on DVE)
  text_embd = (E2_f32r).T @ video_nat  (4 accumulating f32r matmuls) * 1/sum
  video_embd = (EvT slice).T @ text_nat (8 f32r matmuls, K=64 row-packed) * 1/sum
"""

import math

import numpy as np

import concourse.bass as bass
import concourse.mybir as mybir
import concourse.tile as tile
from concourse import bacc
from concourse.bass_utils import run_bass_kernel_spmd
from concourse.masks import make_identity

B, NV, NT, D = 32, 512, 64, 512
NCORES = 8
VPC = B // NCORES          # videos per core = 4
NI = B // 2                # paired-text iterations = 16
P = 128
MC = NV // P               # m chunks = 4
DC = D // P                # d chunks = 4
F32 = mybir.dt.float32
F32R = mybir.dt.float32r

_PROGRAM = None


def _build_program():
    nc = bacc.Bacc(None)

    vns = nc.dram_tensor("vns", [VPC, NV, D], F32, kind="ExternalInput")
    vts = nc.dram_tensor("vts", [VPC, D, NV], F32, kind="ExternalInput")
    tn2 = nc.dram_tensor("tn2", [NI, P, D], F32, kind="ExternalInput")
    tt2 = nc.dram_tensor("tt2", [NI, D, 2 * NT], F32, kind="ExternalInput")
    mkcol = nc.dram_tensor("mkcol", [P, NI], F32, kind="ExternalInput")
    mkrep = nc.dram_tensor("mkrep", [NI, P, 2 * NT], F32, kind="ExternalInput")

    vout = nc.dram_tensor("vout", [VPC, B, NV, D], F32, kind="ExternalOutput")
    tout = nc.dram_tensor("tout", [VPC, B, NT, D], F32, kind="ExternalOutput")
    aout = nc.dram_tensor("aout", [VPC, B, NV, NT], F32, kind="ExternalOutput")
    bout = nc.dram_tensor("bout", [VPC, B, NT, NV], F32, kind="ExternalOutput")

    with tile.TileContext(nc) as tc:
        with (
            tc.tile_pool(name="consts", bufs=1) as consts,
            tc.tile_pool(name="work", bufs=2) as work,
            tc.tile_pool(name="outp", bufs=2) as outp,
            tc.tile_pool(name="scal", bufs=4) as scal,
            tc.tile_pool(name="ps_st", bufs=2, space="PSUM") as ps_st,
            tc.tile_pool(name="ps_e", bufs=2, space="PSUM") as ps_e,
            tc.tile_pool(name="ps_te", bufs=2, space="PSUM") as ps_te,
            tc.tile_pool(name="ps_ve", bufs=2, space="PSUM") as ps_ve,
        ):
            # ---- resident inputs (matmul operands hardware-rounded to f32r) ----
            vns_sb = consts.tile([P, VPC, MC, D], F32R)
            nc.gpsimd.dma_start(
                out=vns_sb, in_=vns.rearrange("v (mc p) d -> p v mc d", p=P)
            )
            vts_sb = consts.tile([P, VPC, DC, NV], F32R)
            nc.gpsimd.dma_start(
                out=vts_sb, in_=vts.rearrange("v (dc p) m -> p v dc m", p=P)
            )
            tn2_sb = consts.tile([P, NI, D], F32R)
            nc.gpsimd.dma_start(out=tn2_sb, in_=tn2.rearrange("i p d -> p i d"))
            tt2_sb = consts.tile([P, NI, DC, 2 * NT], F32R)
            nc.gpsimd.dma_start(
                out=tt2_sb, in_=tt2.rearrange("i (dc p) j -> p i dc j", p=P)
            )
            mkcol_sb = consts.tile([P, NI], F32)
            nc.sync.dma_start(out=mkcol_sb, in_=mkcol[:, :])
            mkrep_sb = consts.tile([P, NI, 2 * NT], F32)
            nc.sync.dma_start(out=mkrep_sb, in_=mkrep.rearrange("i p j -> p i j"))
            ident = consts.tile([P, P], F32)
            make_identity(nc, ident)

            for v in range(VPC):
                for i in range(NI):
                    # ---- scores (transposed layout): ST[(two,n), m] ----
                    ST = ps_st.tile([P, NV], F32, tag="st")
                    for dc in range(DC):
                        nc.tensor.matmul(
                            ST,
                            lhsT=tt2_sb[:, i, dc, :],
                            rhs=vts_sb[:, v, dc, :],
                            start=(dc == 0),
                            stop=(dc == DC - 1),
                        )

                    # ---- exp + fused text-softmax denominators ----
                    ET = work.tile([P, NV], F32, tag="et")
                    stx = scal.tile([P, 1], F32, tag="stx")
                    nc.scalar.activation(
                        ET, ST, mybir.ActivationFunctionType.Exp, accum_out=stx
                    )
                    rtx = scal.tile([P, 1], F32, tag="rtx")
                    nc.vector.reciprocal(rtx, stx)

                    # ---- text_att_scores (full fp32) ----
                    At2 = outp.tile([P, NV], F32, tag="at2")
                    nc.vector.tensor_scalar_mul(At2, ET, rtx)
                    nc.sync.dma_start(
                        out=bout[v, 2 * i : 2 * i + 2].rearrange("t n m -> (t n) m"),
                        in_=At2,
                    )

                    # ---- masked exp (vis path), transposed layout, f32r ----
                    EvT = work.tile([P, NV], F32R, tag="evt")
                    nc.vector.tensor_scalar_mul(EvT, ET, mkcol_sb[:, i : i + 1])

                    # ---- transpose ET -> E2[m_local, mc, (two,n)] in PSUM ----
                    E2ps = ps_e.tile([P, MC, P], F32, tag="e2ps")
                    for mc in range(MC):
                        nc.tensor.transpose(
                            E2ps[:, mc, :], ET[:, mc * P : (mc + 1) * P], ident
                        )
                    E2r = work.tile([P, MC, P], F32R, tag="e2r")
                    nc.vector.tensor_copy(E2r, E2ps)

                    # ---- vis_att_scores numerators (fp32) + row sums ----
                    Evmn = work.tile([P, MC, 2, NT], F32, tag="evmn")
                    nc.vector.tensor_tensor(
                        Evmn,
                        E2ps[:].rearrange("p a (b c) -> p a b c", c=NT),
                        mkrep_sb[:, i, None, :]
                        .rearrange("p a (b c) -> p a b c", c=NT)
                        .to_broadcast([P, MC, 2, NT]),
                        op=mybir.AluOpType.mult,
                    )
                    sv = scal.tile([P, MC, 2], F32, tag="sv")
                    nc.vector.tensor_reduce(
                        sv, Evmn, axis=mybir.AxisListType.X, op=mybir.AluOpType.add
                    )
                    rv = scal.tile([P, MC, 2], F32, tag="rv")
                    nc.vector.reciprocal(rv, sv)

                    Avis = outp.tile([P, MC, 2, NT], F32, tag="avis")
                    nc.vector.tensor_tensor(
                        Avis,
                        Evmn,
                        rv[:, :, :, None].to_broadcast([P, MC, 2, NT]),
                        op=mybir.AluOpType.mult,
                    )
                    for two in range(2):
                        nc.sync.dma_start(
                            out=aout[v, 2 * i + two].rearrange(
                                "(mc p) n -> p mc n", p=P
                            ),
                            in_=Avis[:, :, two, :],
                        )

                    # ---- text_embd_att ----
                    TE = ps_te.tile([P, D], F32, tag="te")
                    for mc in range(MC):
                        nc.tensor.matmul(
                            TE,
                            lhsT=E2r[:, mc, :],
                            rhs=vns_sb[:, v, mc, :],
                            start=(mc == 0),
                            stop=(mc == MC - 1),
                        )
                    te2 = outp.tile([P, D], F32, tag="te2")
                    nc.scalar.activation(
                        te2, TE, mybir.ActivationFunctionType.Copy, scale=rtx
                    )
                    nc.sync.dma_start(
                        out=tout[v, 2 * i : 2 * i + 2].rearrange("t n d -> (t n) d"),
                        in_=te2,
                    )

                    # ---- video_embd_att: 8 K=64 f32r matmuls (row-packed) ----
                    for two in range(2):
                        veb = outp.tile([P, MC, D], F32, tag="vebig")
                        pb = two * NT
                        for mc in range(MC):
                            VE = ps_ve.tile([P, D], F32, tag="ve")
                            nc.tensor.matmul(
                                VE,
                                lhsT=EvT[pb : pb + NT, mc * P : (mc + 1) * P],
                                rhs=tn2_sb[pb : pb + NT, i, :],
                                start=True,
                                stop=True,
                                tile_position=(pb, 0),
                            )
                            rv_s = rv[:, mc, two : two + 1]
                            if mc % 2 == 0:
                                nc.scalar.activation(
                                    veb[:, mc, :],
                                    VE,
                                    mybir.ActivationFunctionType.Copy,
                                    scale=rv_s,
                                )
                            else:
                                nc.vector.tensor_scalar_mul(veb[:, mc, :], VE, rv_s)
                        nc.sync.dma_start(
                            out=vout[v, 2 * i + two].rearrange(
                                "(mc p) d -> p mc d", p=P
                            ),
                            in_=veb,
                        )
    nc.finalize()
    return nc


def _get_program():
    global _PROGRAM
    if _PROGRAM is None:
        _PROGRAM = _build_program()
    return _PROGRAM


def _prep_in_maps(video_embd, text_embd, mask):
    video = np.ascontiguousarray(np.asarray(video_embd, dtype=np.float32))
    text = np.ascontiguousarray(np.asarray(text_embd, dtype=np.float32))
    maskf = np.asarray(mask).astype(np.float32)

    scale = np.float32(1.0) / np.sqrt(np.float32(D))

    tn2 = np.ascontiguousarray(text.reshape(NI, P, D))
    tt2 = np.ascontiguousarray(
        text.reshape(NI, 2, NT, D).transpose(0, 3, 1, 2).reshape(NI, D, 2 * NT)
    )
    mkcol = np.ascontiguousarray(maskf.reshape(NI, 2 * NT).T)
    mkrep = np.ascontiguousarray(
        np.broadcast_to(maskf.reshape(NI, 1, 2 * NT), (NI, P, 2 * NT))
    )

    in_maps = []
    for c in range(NCORES):
        sl = video[c * VPC : (c + 1) * VPC]
        in_maps.append(
            {
                "vns": sl,
                "vts": np.ascontiguousarray(sl.transpose(0, 2, 1)) * scale,
                "tn2": tn2,
                "tt2": tt2,
                "mkcol": mkcol,
                "mkrep": mkrep,
            }
        )
    return in_maps


def _run(video_embd, text_embd, mask, trace=False):
    nc = _get_program()
    in_maps = _prep_in_maps(video_embd, text_embd, mask)
    out = run_bass_kernel_spmd(nc, in_maps, list(range(NCORES)), trace=trace)
    res = out.results
    video_embd_att = np.concatenate([res[c]["vout"] for c in range(NCORES)], axis=0)
    text_embd_att = np.concatenate([res[c]["tout"] for c in range(NCORES)], axis=0)
    vis_att_scores = np.concatenate([res[c]["aout"] for c in range(NCORES)], axis=0)
    text_att_scores = np.concatenate([res[c]["bout"] for c in range(NCORES)], axis=0)
    return (
        (video_embd_att, text_embd_att, vis_att_scores, text_att_scores),
        out.exec_time_ns,
    )


def kernel(video_embd, text_embd, mask):
    outs, _ = _run(video_embd, text_embd, mask, trace=False)
    return outs
